# revision 3
# baseline (speedup 1.0000x reference)
"""Trainium2 Bass kernel for nn_BCNet: three-way low-rank bilinear net.

reference:
  v_ = relu(v @ Wv.T + bv)            # (B, NV, HK)
  q_ = relu(q @ Wq.T + bq)            # (B, NQ, HK)
  logits = einsum('hk,bvk,bqk->bhvq', h_mat, v_, q_) + h_bias

Sharding: data-parallel over batch, 4 batch items per core (8 cores).
All matmuls in bf16 with fp32 PSUM accumulation.

Host prep per core:
  vT   (4, 2048, 512) bf16  : v[b].T per batch item
  qT   (1024, 512)    bf16  : q[4c:4c+4] transposed+stacked, cols = b*128+q
  WvT  (2048, 1536)   bf16
  WqT  (1024, 1536)   bf16
  bvT  (128, 12) f32 : bv[jc*128+p]
  bqT  (128, 12) f32
  hm   (128, 12, 8) f32 : h_mat[h, jc*128+p]
  hb   (128, 8) f32 : h_bias[h] broadcast over partitions
Device output per core: out (4, 8, 128, 512) bf16 = logits[b, h, q, v].
Host post: concat -> (32, 8, 128, 512) -> f32 -> transpose -> (32, 8, 512, 128).

Schedule notes (from trace analysis):
- Weight/input DMAs are issued as 512-column blocks in exact consumption
  order, split across the sync and gpsimd issue streams: per-queue DMA
  bandwidth (~90 GB/s) makes a full-width 393KB chunk take ~4.3us, which
  throttled stage B.  131KB blocks across parallel queues keep stage B
  PE-bound instead of arrival-bound.
- A few dummy matmuls on a memset tile spin up the PE clock (0.65 ->
  1.2 -> 2.4 GHz ramp) before the first real operand lands.
- Stage C emits one PSUM tile [128 q, 512 v] per (b, h): the h_bias add is
  a per-partition-scalar Identity activation (bias constant within an h
  tile), cast to bf16 on the way out (halves store traffic).
- PSUM: stage A/B ring = 6 banks, stage C ring = 2; a group's first matmul
  lands in a bank freed more than a full group earlier, so the ReLU drain
  of the previous group is never on the PE critical path.
- vT[b] for b>=1 loads as one 3D DMA (2.1MB) issued from the sync stream
  right after stage A(b-1)'s matmuls, long before A(b) needs it.
- The final store is split in half across the two issue engines so the
  drain/barrier epilogue starts ~1us earlier.
"""

import numpy as np

B, NV, NQ = 32, 512, 128
V_DIM, Q_DIM, HK, H_OUT = 2048, 1024, 1536, 8
N_CORES = 8
BPC = B // N_CORES          # 4 batch items per core
JC = HK // 128              # 12 k-chunks
DCV = V_DIM // 128          # 16 contraction chunks for v
DCQ = Q_DIM // 128          # 8 contraction chunks for q

_CACHE = {}


def _build_nc():
    import concourse.tile as tile
    from concourse import bacc, mybir
    from contextlib import ExitStack

    bf16 = mybir.dt.bfloat16
    f32 = mybir.dt.float32

    nc = bacc.Bacc()

    vT = nc.declare_dram_parameter("vT", [BPC, V_DIM, NV], bf16, isOutput=False)
    qT = nc.declare_dram_parameter("qT", [Q_DIM, BPC * NQ], bf16, isOutput=False)
    WvT = nc.declare_dram_parameter("WvT", [V_DIM, HK], bf16, isOutput=False)
    WqT = nc.declare_dram_parameter("WqT", [Q_DIM, HK], bf16, isOutput=False)
    bvT = nc.declare_dram_parameter("bvT", [128, JC], f32, isOutput=False)
    bqT = nc.declare_dram_parameter("bqT", [128, JC], f32, isOutput=False)
    hm = nc.declare_dram_parameter("hm", [128, JC, H_OUT], f32, isOutput=False)
    hb = nc.declare_dram_parameter("hb", [128, H_OUT], f32, isOutput=False)
    out = nc.declare_dram_parameter("out", [BPC, H_OUT, NQ, NV], bf16, isOutput=True)

    with ExitStack() as ctx:
        tc = ctx.enter_context(tile.TileContext(nc))
        consts = ctx.enter_context(tc.tile_pool(name="consts", bufs=1))
        qpool = ctx.enter_context(tc.tile_pool(name="qpool", bufs=1))
        vin = ctx.enter_context(tc.tile_pool(name="vin", bufs=2))
        vact = ctx.enter_context(tc.tile_pool(name="vact", bufs=2))
        qhp = ctx.enter_context(tc.tile_pool(name="qhp", bufs=1))
        ocp = ctx.enter_context(tc.tile_pool(name="ocp", bufs=4))
        psAB = ctx.enter_context(tc.tile_pool(name="psAB", bufs=6, space="PSUM"))
        psC = ctx.enter_context(tc.tile_pool(name="psC", bufs=2, space="PSUM"))

        # ---- PE clock warmup: a few throwaway matmuls on a zeroed tile so
        # the tensor engine's frequency ramp runs during the DMA fill.
        warm_sb = consts.tile([128, NV], bf16)
        nc.vector.memset(warm_sb, 0.0)
        warm_ps = psAB.tile([128, NV], f32, tag="ps", name="warm_ps")
        for _ in range(4):
            nc.tensor.matmul(
                warm_ps, lhsT=warm_sb[:, 0:128], rhs=warm_sb, start=True, stop=True
            )

        # ---- inputs, issued as 512-col blocks in consumption order.
        # sync:   qt 0..7 | wq blk1 | (wv blk0 + vt0) pairs | wv blk2
        # gpsimd: wq blk0 | bq | wq blk2 | bv hm hb | wv blk1
        qT_r = qT.rearrange("(d p) n -> p d n", p=128)
        qt_sb = qpool.tile([128, DCQ, BPC * NQ], bf16)
        WqT_r = WqT.rearrange("(d p) j -> p d j", p=128)
        wq_sb = consts.tile([128, DCQ, HK], bf16)
        for dd in range(DCQ):
            nc.sync.dma_start(out=qt_sb[:, dd, :], in_=qT_r[:, dd, :])
            nc.gpsimd.dma_start(
                out=wq_sb[:, dd, 0:512], in_=WqT_r[:, dd, 0:512]
            )
        bq_sb = consts.tile([128, JC], f32)
        nc.gpsimd.dma_start(out=bq_sb, in_=bqT[:, :])
        for dd in range(DCQ):
            nc.sync.dma_start(out=wq_sb[:, dd, 512:1024], in_=WqT_r[:, dd, 512:1024])
            nc.gpsimd.dma_start(out=wq_sb[:, dd, 1024:1536], in_=WqT_r[:, dd, 1024:1536])
        bv_sb = consts.tile([128, JC], f32)
        nc.gpsimd.dma_start(out=bv_sb, in_=bvT[:, :])
        hm_sb = consts.tile([128, JC, H_OUT], f32)
        nc.gpsimd.dma_start(out=hm_sb, in_=hm[:, :, :])
        hb_sb = consts.tile([128, H_OUT], f32)
        nc.gpsimd.dma_start(out=hb_sb, in_=hb[:, :])
        # stage A feed: (wv blk0[d], vt0[d]) pairs on sync in consumption
        # order; wv blk1 on gpsimd; wv blk2 on sync after.
        WvT_r = WvT.rearrange("(d p) j -> p d j", p=128)
        wv_sb = consts.tile([128, DCV, HK], bf16)
        vt0_sb = vin.tile([128, DCV, NV], bf16, tag="vt", name="vt0")
        vT0_r = vT[0].rearrange("(d p) n -> p d n", p=128)
        for dd in range(DCV):
            nc.sync.dma_start(out=wv_sb[:, dd, 0:512], in_=WvT_r[:, dd, 0:512])
            nc.sync.dma_start(out=vt0_sb[:, dd, :], in_=vT0_r[:, dd, :])
        for dd in range(DCV):
            nc.gpsimd.dma_start(out=wv_sb[:, dd, 512:1024], in_=WvT_r[:, dd, 512:1024])
        for dd in range(DCV):
            nc.sync.dma_start(out=wv_sb[:, dd, 1024:1536], in_=WvT_r[:, dd, 1024:1536])

        # ---- stage B: q_ = relu(q @ Wq.T + bq), all 4 b at once ----
        # d-outer within groups of 4 j's: weight block d is consumed ~4
        # matmuls after block d-1, matching DMA arrival order.
        qact_sb = qpool.tile([128, JC, BPC * NQ], bf16)
        for jg in range(0, JC, 4):
            pss = [psAB.tile([128, BPC * NQ], f32, tag="ps", name=f"psB{jg}_{i}") for i in range(4)]
            for d in range(DCQ):
                for ji in range(4):
                    j = jg + ji
                    nc.tensor.matmul(
                        pss[ji],
                        lhsT=wq_sb[:, d, j * 128:(j + 1) * 128],
                        rhs=qt_sb[:, d, :],
                        start=(d == 0),
                        stop=(d == DCQ - 1),
                    )
            for ji in range(4):
                j = jg + ji
                nc.scalar.activation(
                    out=qact_sb[:, j, :],
                    in_=pss[ji],
                    func=mybir.ActivationFunctionType.Relu,
                    bias=bq_sb[:, j:j + 1],
                    scale=1.0,
                )

        vt_next = None
        for b in range(BPC):
            # ---- build Qh[b][k, h, q'] = q_[k, b*128+q'] * h_mat[h, k] (DVE)
            qh_sb = qhp.tile([128, JC, H_OUT, NQ], bf16, tag="qh")
            for j in range(JC):
                for h in range(H_OUT):
                    nc.vector.tensor_scalar_mul(
                        qh_sb[:, j, h, :],
                        qact_sb[:, j, b * NQ:(b + 1) * NQ],
                        hm_sb[:, j, h:h + 1],
                    )

            # ---- stage A: v_[b] = relu(v[b] @ Wv.T + bv), transposed layout
            vt_sb = vt0_sb if b == 0 else vt_next
            vact_sb = vact.tile([128, JC, NV], bf16, tag="vact")
            for jg in range(0, JC, 4):
                pss = [psAB.tile([128, NV], f32, tag="ps", name=f"psA{b}_{jg}_{i}") for i in range(4)]
                for d in range(DCV):
                    for ji in range(4):
                        j = jg + ji
                        nc.tensor.matmul(
                            pss[ji],
                            lhsT=wv_sb[:, d, j * 128:(j + 1) * 128],
                            rhs=vt_sb[:, d, :],
                            start=(d == 0),
                            stop=(d == DCV - 1),
                        )
                for ji in range(4):
                    j = jg + ji
                    nc.scalar.activation(
                        out=vact_sb[:, j, :],
                        in_=pss[ji],
                        func=mybir.ActivationFunctionType.Relu,
                        bias=bv_sb[:, j:j + 1],
                        scale=1.0,
                    )

            # prefetch vT[b+1] as a single 3D DMA; emitted here so the sync
            # stream issues it before this b's output stores.
            if b + 1 < BPC:
                vt_next = vin.tile([128, DCV, NV], bf16, tag="vt", name=f"vt{b+1}")
                nc.sync.dma_start(
                    out=vt_next, in_=vT[b + 1].rearrange("(d p) n -> p d n", p=128)
                )

            # ---- stage C: logits[b, h] = (Qh[b, :, h].T @ v_[b].T).T
            # One [128 q, 512 v] PSUM tile per h; h_bias[h] is constant
            # within the tile so the bias add rides the Identity activation
            # that drains PSUM to bf16 SBUF. Stores alternate issue engines.
            for h in range(H_OUT):
                po = psC.tile([128, NV], f32, tag="psc", name=f"psC{b}_{h}")
                for j in range(JC):
                    nc.tensor.matmul(
                        po,
                        lhsT=qh_sb[:, j, h, :],
                        rhs=vact_sb[:, j, :],
                        start=(j == 0),
                        stop=(j == JC - 1),
                    )
                oc_sb = ocp.tile([128, NV], bf16, tag="oc", name=f"oc{b}_{h}")
                last = b == BPC - 1 and h == H_OUT - 1
                if not last:
                    nc.scalar.activation(
                        out=oc_sb,
                        in_=po,
                        func=mybir.ActivationFunctionType.Identity,
                        bias=hb_sb[:, h:h + 1],
                        scale=1.0,
                    )
                    eng = nc.sync if h % 2 == 0 else nc.gpsimd
                    eng.dma_start(out=out[b, h, :, :], in_=oc_sb)
                else:
                    # split the final tile so both halves drain in parallel
                    # and the epilogue barrier isn't gated on one long store
                    for half, eng in ((0, nc.sync), (1, nc.gpsimd)):
                        sl = slice(half * 256, (half + 1) * 256)
                        nc.scalar.activation(
                            out=oc_sb[:, sl],
                            in_=po[:, sl],
                            func=mybir.ActivationFunctionType.Identity,
                            bias=hb_sb[:, h:h + 1],
                            scale=1.0,
                        )
                        eng.dma_start(out=out[b, h, :, sl], in_=oc_sb[:, sl])

    nc.compile()
    return nc


def kernel(v, q, Wv, bv, Wq, bq, h_mat, h_bias):
    import ml_dtypes
    from concourse import bass_utils

    bf16 = ml_dtypes.bfloat16

    if "nc" not in _CACHE:
        _CACHE["nc"] = _build_nc()
    nc = _CACHE["nc"]

    v = np.asarray(v, dtype=np.float32)
    q = np.asarray(q, dtype=np.float32)
    Wv = np.asarray(Wv, dtype=np.float32)
    Wq = np.asarray(Wq, dtype=np.float32)
    bv = np.asarray(bv, dtype=np.float32)
    bq = np.asarray(bq, dtype=np.float32)
    h_mat = np.asarray(h_mat, dtype=np.float32)
    h_bias = np.asarray(h_bias, dtype=np.float32)

    vT = np.ascontiguousarray(v.transpose(0, 2, 1)).astype(bf16)      # (B, 2048, 512)
    WvT = np.ascontiguousarray(Wv.T).astype(bf16)                     # (2048, 1536)
    WqT = np.ascontiguousarray(Wq.T).astype(bf16)                     # (1024, 1536)
    bvT = np.ascontiguousarray(bv.reshape(JC, 128).T)                 # (128, 12)
    bqT = np.ascontiguousarray(bq.reshape(JC, 128).T)
    # hm[p, jc, h] = h_mat[h, jc*128+p]
    hmP = np.ascontiguousarray(h_mat.reshape(H_OUT, JC, 128).transpose(2, 1, 0))
    hbB = np.ascontiguousarray(np.broadcast_to(h_bias[None, :], (128, H_OUT)))

    in_maps = []
    for c in range(N_CORES):
        bs = slice(BPC * c, BPC * (c + 1))
        qTc = np.ascontiguousarray(
            q[bs].transpose(2, 0, 1).reshape(Q_DIM, BPC * NQ)
        ).astype(bf16)
        in_maps.append({
            "vT": vT[bs],
            "qT": qTc,
            "WvT": WvT,
            "WqT": WqT,
            "bvT": bvT,
            "bqT": bqT,
            "hm": hmP,
            "hb": hbB,
        })

    res = bass_utils.run_bass_kernel_spmd(nc, in_maps, list(range(N_CORES)))
    outs = np.concatenate([res.results[c]["out"] for c in range(N_CORES)], axis=0)
    # (32, 8, 128, 512) bf16 -> f32 -> (32, 8, 512, 128)
    logits = outs.astype(np.float32).transpose(0, 1, 3, 2)
    return np.ascontiguousarray(logits)
